# revision 28
# baseline (speedup 1.0000x reference)
"""Trainium2 Bass kernel for the MAB dense-transformer block (fp8 version).

Sharding: 8 cores = 2 batches x 4 Sq-slices (512 each). Each core:
  - projects k (bf16 out) and vT (fp8) for its whole batch (replicated across
    the 4 cores sharing the batch; hidden under the ACT-bound attention phase),
  - q = wq @ Q_b[:, slice] (bf16 out),
  - per head: logits^T = k_h^T q_h in bf16 ([Sk, Sq_loc] tiles), sigmoid on ACT
    in 1536-wide batches with the dk^-0.5 scale folded into the ACT scale,
    o^T accumulated via fp8 DoubleRow matmuls with a 0.25-column in vT giving
    row-sums for the renormalization, division via gpsimd broadcast of 1/s,
  - proj (wp, fp8) + Q residual, FFN in bf16 (fp8 there dominates output
    error; the attention path's fp8 error largely cancels in the renorm).
Attention-phase matmuls are fp8e4 DoubleRow (2x PE throughput) with fp32 PSUM.
Weights are pre-scaled by 8 on the host so fp8 stays clear of subnormals; the
inverse scales fold into ACT scale params and the renorm column. The sigmoid
(ScalarE) is the attention-phase bottleneck, so K/V/Q projection work is
interleaved at fine grain between sigmoid groups to keep the PE busy (HAM
re-throttles the PE clock to 1.2 GHz after ~3.4us of idle).
"""

import numpy as np
import ml_dtypes

BF = ml_dtypes.bfloat16
F8 = ml_dtypes.float8_e4m3  # TRN float8e4: bias 7, max 240

B, DIM, H, DK, SQ, SK = 2, 1024, 16, 64, 2048, 2048
D = H * DK
NCORES = 8
QSL = SQ // 4          # 512 columns of Sq per core
NG = 4                 # head groups per core (4 heads each)

WS = 8.0               # host-side weight upscale (fp8 weight matrices)
ONES_VAL = 0.25        # vT ones-column (v scaled x4) -> o_sb = 16 * o_true
VSC = 0.5              # vt = 4*v_raw = (8*v_raw_psum) * 0.5
SIG_SCALE = 1.0 / 512  # 1/(WS_q*WS_k*sqrt(DK))
PROJ_SCALE = 1.0 / 128 # 1/(WS_p*16)

_nc_cache = {}
_host_cache = {}


def _build_nc(mask_ones, bq_nz, bk_nz, bp_nz, b2_nz):
    from concourse import bacc, mybir
    import concourse.tile as tile

    bf16 = mybir.dt.bfloat16
    f8 = mybir.dt.float8e4
    f32 = mybir.dt.float32
    AF = mybir.ActivationFunctionType
    DR = mybir.MatmulPerfMode.DoubleRow

    nc = bacc.Bacc("TRN2")

    # activation inputs are partition-major on the host for wide DMA rows
    d_Kb = nc.declare_dram_parameter("Kb", [128, 8, SK], f8, isOutput=False)
    d_Qb = nc.declare_dram_parameter("Qb", [128, 8, QSL], f8, isOutput=False)
    d_Qres = nc.declare_dram_parameter("Qres", [128, 8, QSL], f32, isOutput=False)
    d_wq = nc.declare_dram_parameter("wq", [8, 128, 8, 128], f8, isOutput=False)
    d_wk = nc.declare_dram_parameter("wk", [8, 128, 8, 128], f8, isOutput=False)
    d_wv = nc.declare_dram_parameter("wv", [8, 128, D], f8, isOutput=False)
    d_wp = nc.declare_dram_parameter("wp", [8, 128, 8, 128], f8, isOutput=False)
    d_w1 = nc.declare_dram_parameter("w1", [16, 128, 8, 128], bf16, isOutput=False)
    d_w2 = nc.declare_dram_parameter("w2", [8, 128, 16, 128], bf16, isOutput=False)
    d_b1 = nc.declare_dram_parameter("b1t", [128, 16], f32, isOutput=False)
    d_bq = d_bk = d_bp = d_b2 = d_madd = None
    if bq_nz:
        d_bq = nc.declare_dram_parameter("bqt", [128, 8], f32, isOutput=False)
    if bk_nz:
        d_bk = nc.declare_dram_parameter("bkt", [128, 8], f32, isOutput=False)
    if bp_nz:
        d_bp = nc.declare_dram_parameter("bpt", [128, 8], f32, isOutput=False)
    if b2_nz:
        d_b2 = nc.declare_dram_parameter("b2t", [128, 8], f32, isOutput=False)
    if not mask_ones:
        d_madd = nc.declare_dram_parameter("maddt", [128, 16], f32, isOutput=False)
    d_out = nc.declare_dram_parameter("out", [DIM, QSL], f32, isOutput=True)

    with tile.TileContext(nc) as tc:
        with (
            tc.tile_pool(name="pin", bufs=1) as pin,
            tc.tile_pool(name="pw", bufs=6) as pw,
            tc.tile_pool(name="pkg", bufs=2) as pkg,
            tc.tile_pool(name="pvt", bufs=2) as pvt,
            tc.tile_pool(name="pq", bufs=1) as pq,
            tc.tile_pool(name="pwt", bufs=3) as pwt,
            tc.tile_pool(name="po", bufs=1) as po,
            tc.tile_pool(name="ph", bufs=1) as ph,
            tc.tile_pool(name="psmall", bufs=2) as psmall,
            tc.tile_pool(name="pqr", bufs=8) as pqr,
            tc.tile_pool(name="pconst", bufs=1) as pconst,
            tc.tile_pool(name="pout", bufs=2) as pout,
            tc.tile_pool(name="ppsL", bufs=2, space="PSUM") as ppsL,
            tc.tile_pool(name="ppsO", bufs=2, space="PSUM") as ppsO,
        ):
            # ---- input loads (qb first so q-proj starts immediately) ----
            qb = pin.tile([128, 8, QSL], f8, tag="qb")
            nc.sync.dma_start(out=qb, in_=d_Qb[:])

            b1_sb = pconst.tile([128, 16], f32, tag="b1")
            nc.sync.dma_start(out=b1_sb, in_=d_b1[:])
            bq_sb = bk_sb = bp_sb = b2_sb = madd_sb = None
            if bq_nz:
                bq_sb = pconst.tile([128, 8], f32, tag="bq")
                nc.sync.dma_start(out=bq_sb, in_=d_bq[:])
            if bk_nz:
                bk_sb = pconst.tile([128, 8], f32, tag="bk")
                nc.sync.dma_start(out=bk_sb, in_=d_bk[:])
            if bp_nz:
                bp_sb = pconst.tile([128, 8], f32, tag="bp")
                nc.sync.dma_start(out=bp_sb, in_=d_bp[:])
            if b2_nz:
                b2_sb = pconst.tile([128, 8], f32, tag="b2")
                nc.sync.dma_start(out=b2_sb, in_=d_b2[:])
            if not mask_ones:
                madd_sb = pconst.tile([128, 16], f32, tag="madd")
                nc.sync.dma_start(out=madd_sb, in_=d_madd[:])

            q_sb = pq.tile([128, 8, QSL], bf16, tag="q")

            def q_unit(m):
                wt = pw.tile([128, 8, 128], f8, tag="w")
                nc.sync.dma_start(out=wt, in_=d_wq[m])
                ps = ppsL.tile([128, QSL], f32, tag="fl")
                for i in range(4):
                    nc.tensor.matmul(
                        ps, wt[:, 2 * i : 2 * i + 2, :], qb[:, 2 * i : 2 * i + 2, :],
                        start=(i == 0), stop=(i == 3), perf_mode=DR,
                    )
                if bq_nz:
                    nc.scalar.activation(
                        q_sb[:, m, :], ps, AF.Identity, bias=bq_sb[:, m : m + 1]
                    )
                else:
                    nc.vector.tensor_copy(q_sb[:, m, :], ps)

            q_unit(0)

            kb = pin.tile([128, 8, SK], f8, tag="kb")
            for half in range(2):
                for c in range(8):
                    nc.sync.dma_start(
                        out=kb[:, c, 1024 * half : 1024 * half + 1024],
                        in_=d_Kb[:, c, 1024 * half : 1024 * half + 1024],
                    )
            wv_sb = pin.tile([128, 8, D], f8, tag="wv")
            for c in range(8):
                nc.sync.dma_start(out=wv_sb[:, c, :], in_=d_wv[c])

            o_sb = po.tile([128, 8, QSL], f8, tag="o")
            o_ff = po.tile([128, 8, QSL], bf16, tag="off")
            o_res = po.tile([128, 8, QSL], f32, tag="ores")

            def emit_kproj(g):
                """kg rows for heads 4g..4g+3 (bf16, x8); fine-grained units."""
                kg = pkg.tile([128, 2, SK], bf16, tag="kg")
                wts = [None, None]

                def k_sub(lm, n):
                    if wts[lm] is None:
                        wt = pw.tile([128, 8, 128], f8, tag="w")
                        nc.sync.dma_start(out=wt, in_=d_wk[2 * g + lm])
                        wts[lm] = wt
                    wt = wts[lm]
                    ps = ppsL.tile([128, 512], f32, tag="fl")
                    for i in range(4):
                        nc.tensor.matmul(
                            ps,
                            wt[:, 2 * i : 2 * i + 2, :],
                            kb[:, 2 * i : 2 * i + 2, 512 * n : 512 * n + 512],
                            start=(i == 0), stop=(i == 3), perf_mode=DR,
                        )
                    if bk_nz:
                        nc.scalar.activation(
                            kg[:, lm, 512 * n : 512 * n + 512], ps, AF.Identity,
                            bias=bk_sb[:, 2 * g + lm : 2 * g + lm + 1],
                        )
                    else:
                        nc.vector.tensor_copy(kg[:, lm, 512 * n : 512 * n + 512], ps)

                units = [lambda lm=lm, n=n: k_sub(lm, n)
                         for lm in range(2) for n in range(4)]
                return kg, units

            def emit_vproj(half):
                """vT (fp8, x4 with 0.25-col at 64) for heads 8h..8h+7."""
                vt = pvt.tile([128, 16, 8, 68], f8, tag="vt")

                def v_sub(t):
                    ps = ppsL.tile([128, 512], f32, tag="fl")
                    for i in range(4):
                        nc.tensor.matmul(
                            ps,
                            kb[:, 2 * i : 2 * i + 2, 128 * t : 128 * t + 128],
                            wv_sb[:, 2 * i : 2 * i + 2, 512 * half : 512 * half + 512],
                            start=(i == 0), stop=(i == 3), perf_mode=DR,
                        )
                    nc.vector.tensor_scalar_mul(
                        vt[:, t, :, 0:64],
                        ps.rearrange("p (h d) -> p h d", h=8), VSC,
                    )

                units = [lambda: nc.vector.memset(vt[:, :, :, 64:65], ONES_VAL)]
                units += [lambda t=t: v_sub(t) for t in range(16)]
                return vt, units

            def norm_tail(ps_o, r0, oc):
                sc = psmall.tile([1, QSL], f32, tag="sc")
                nc.vector.tensor_copy(sc, ps_o[64:65, :])
                rc = psmall.tile([1, QSL], f32, tag="rc")
                nc.vector.reciprocal_approx_fast(out=rc, in_=sc)
                rb = psmall.tile([64, QSL], f32, tag="rb")
                nc.gpsimd.partition_broadcast(rb, rc)
                nc.vector.tensor_mul(o_sb[r0 : r0 + 64, oc, :], ps_o[0:64, :], rb)

            GROUPS = [(2 * t, 2) for t in range(8)]

            def emit_head_pair(kg, vt, g, j, fills):
                """heads 4g+2j (rows 0:64) and 4g+2j+1 (rows 64:128).
                Logits bf16 with E/O on distinct PE row-groups (concurrent);
                sigmoid in 1536-wide ACTs into per-head fp8 wt tiles; o^T via
                fp8 DoubleRow matmuls trailing one sigmoid group. `fills` is a
                list of PE work emitted between sigmoid groups to keep HAM
                warm; all of it is consumed within this pair (deadline)."""
                per = (len(fills) + 7) // 8 if fills else 0
                lm = j
                oc = 2 * g + j
                hE = 4 * (g % 2) + 2 * j
                hO = hE + 1
                wtE = pwt.tile([128, 16, 512], f8, tag="wt")
                wtO = pwt.tile([128, 16, 512], f8, tag="wt")
                ps_oE = ppsO.tile([65, QSL], f32, tag="oacc")
                ps_oO = ppsO.tile([65, QSL], f32, tag="oacc")
                state = {"omm": 0}

                def emit_omms(upto):
                    while state["omm"] < upto:
                        i = state["omm"]
                        nc.tensor.matmul(
                            ps_oE, vt[:, 2 * i : 2 * i + 2, hE, 0:65],
                            wtE[:, 2 * i : 2 * i + 2, :],
                            start=(i == 0), stop=(i == 7), perf_mode=DR,
                        )
                        nc.tensor.matmul(
                            ps_oO, vt[:, 2 * i : 2 * i + 2, hO, 0:65],
                            wtO[:, 2 * i : 2 * i + 2, :],
                            start=(i == 0), stop=(i == 7), perf_mode=DR,
                        )
                        state["omm"] += 1

                avail = 0
                for gi, (t0, w) in enumerate(GROUPS):
                    ps_lE = ppsL.tile([128, 512 * w], f32, tag="lg")
                    ps_lO = ppsL.tile([128, 512 * w], f32, tag="lg")
                    for dt_ in range(w):
                        t = t0 + dt_
                        nc.tensor.matmul(
                            ps_lE[:, 512 * dt_ : 512 * dt_ + 512],
                            kg[0:64, lm, 128 * t : 128 * t + 128],
                            q_sb[0:64, oc, :], start=True, stop=True,
                        )
                        nc.tensor.matmul(
                            ps_lO[:, 512 * dt_ : 512 * dt_ + 512],
                            kg[64:128, lm, 128 * t : 128 * t + 128],
                            q_sb[64:128, oc, :], start=True, stop=True,
                        )
                    # PE filler after the next sigmoid's logits so the PE
                    # chain to the sigmoid input stays short (HAM stays warm)
                    for _ in range(per):
                        if fills:
                            fills.pop(0)()
                    emit_omms(avail // 2)
                    if mask_ones:
                        nc.scalar.activation(
                            wtE[:, t0 : t0 + w, :], ps_lE, AF.Sigmoid,
                            scale=SIG_SCALE,
                        )
                        nc.scalar.activation(
                            wtO[:, t0 : t0 + w, :], ps_lO, AF.Sigmoid,
                            scale=SIG_SCALE,
                        )
                    else:
                        for dt_ in range(w):
                            t = t0 + dt_
                            for ps_l, wt_t in ((ps_lE, wtE), (ps_lO, wtO)):
                                nc.scalar.activation(
                                    wt_t[:, t, :],
                                    ps_l[:, 512 * dt_ : 512 * dt_ + 512],
                                    AF.Sigmoid, scale=SIG_SCALE,
                                    bias=madd_sb[:, t : t + 1],
                                )
                    avail = t0 + w
                while fills:
                    fills.pop(0)()
                emit_omms(8)
                norm_tail(ps_oE, 0, oc)
                norm_tail(ps_oO, 64, oc)

            # ---- prologue: minimal work before pair 0 ----
            kg0, k0u = emit_kproj(0)
            vt0, v0u = emit_vproj(0)
            for u in k0u[0:4]:      # kg g0 lm=0 (pair 0)
                u()
            v0u[0]()                # ones memset
            for u in v0u[1:5]:      # vt half0 t0..3
                u()
            q_unit(1)

            kg1, k1u = emit_kproj(1)
            qr_tiles = []
            kgs = {0: kg0, 1: kg1}
            kus = {0: k0u, 1: k1u}
            vts = {0: vt0}
            vus = {0: v0u}

            # fill schedule per pair; every unit must land before its reader:
            #   kg g lm read by pair 2g+lm; vt half-h chunk t by pair 4h+; a
            #   pair's own vt chunks t by its sigmoid-group t/2 + 1
            for g in range(NG):
                for j in range(2):
                    p = 2 * g + j
                    if p == 0:
                        plan = k0u[4:8] + v0u[5:17]          # kg0 lm1; v0 t4-15
                    elif p == 1:
                        plan = k1u[0:4] + [lambda: q_unit(2), lambda: q_unit(3)]
                    elif p == 2:
                        vt1, v1u = emit_vproj(1)
                        vts[1] = vt1
                        vus[1] = v1u
                        plan = k1u[4:8] + v1u[0:5] + [lambda: q_unit(4)]
                    elif p == 3:
                        kg2, k2u = emit_kproj(2)
                        kgs[2] = kg2
                        kus[2] = k2u
                        plan = vus[1][5:13] + k2u[0:4]       # v1 t4-11; kg2 lm0
                    elif p == 4:
                        plan = vus[1][13:17] + kus[2][4:8] \
                            + [lambda: q_unit(5), lambda: q_unit(6)]
                    elif p == 5:
                        kg3, k3u = emit_kproj(3)
                        kgs[3] = kg3
                        kus[3] = k3u
                        plan = k3u[0:4] + [lambda: q_unit(7)]
                    elif p == 6:
                        plan = kus[3][4:8]
                        # prefetch Q residual during remaining attention
                        for m in range(8):
                            qr = pqr.tile([128, QSL], f32, tag="qr")
                            nc.sync.dma_start(out=qr, in_=d_Qres[:, m, :])
                            qr_tiles.append(qr)
                    else:
                        plan = []
                    emit_head_pair(kgs[g], vts[g // 2], g, j, plan)

            # ---- proj + Q residual (fp8 DoubleRow; o_sb error cancels) ----
            for m in range(8):
                wt = pw.tile([128, 8, 128], f8, tag="w")
                nc.sync.dma_start(out=wt, in_=d_wp[m])
                ps = ppsL.tile([128, QSL], f32, tag="lg")
                for i in range(4):
                    nc.tensor.matmul(
                        ps, wt[:, 2 * i : 2 * i + 2, :], o_sb[:, 2 * i : 2 * i + 2, :],
                        start=(i == 0), stop=(i == 3), perf_mode=DR,
                    )
                tp = psmall.tile([128, QSL], f32, tag="tp")
                if bp_nz:
                    nc.scalar.activation(
                        tp, ps, AF.Identity, scale=PROJ_SCALE,
                        bias=bp_sb[:, m : m + 1],
                    )
                else:
                    nc.scalar.activation(tp, ps, AF.Identity, scale=PROJ_SCALE)
                qr = qr_tiles[m]
                nc.vector.tensor_add(o_res[:, m, :], tp, qr)
                nc.vector.tensor_copy(o_ff[:, m, :], o_res[:, m, :])

            # ---- FFN (bf16 for precision: fp8 here dominates output error) ----
            h_sb = ph.tile([128, 16, QSL], bf16, tag="h")
            for m in range(16):
                wt = pw.tile([128, 8, 128], bf16, tag="wb", bufs=4)
                nc.sync.dma_start(out=wt, in_=d_w1[m])
                ps = ppsL.tile([128, QSL], f32, tag="lg")
                for c in range(8):
                    nc.tensor.matmul(
                        ps, wt[:, c, :], o_ff[:, c, :],
                        start=(c == 0), stop=(c == 7),
                    )
                nc.scalar.activation(
                    h_sb[:, m, :], ps, AF.Relu, bias=b1_sb[:, m : m + 1]
                )
            for m in range(8):
                wt = pw.tile([128, 16, 128], bf16, tag="wb", bufs=4)
                nc.sync.dma_start(out=wt, in_=d_w2[m])
                ps = ppsL.tile([128, QSL], f32, tag="lg")
                for c in range(16):
                    nc.tensor.matmul(
                        ps, wt[:, c, :], h_sb[:, c, :],
                        start=(c == 0), stop=(c == 15),
                    )
                if b2_nz:
                    nc.scalar.activation(
                        ps, ps, AF.Identity, bias=b2_sb[:, m : m + 1]
                    )
                ot = pout.tile([128, QSL], f32, tag="out")
                nc.vector.tensor_add(ot, ps, o_res[:, m, :])
                nc.sync.dma_start(out=d_out[128 * m : 128 * m + 128, :], in_=ot)

    nc.finalize()
    return nc


def _tile_lhsT(wT, mt, ct):
    # wT [K, M] -> [M/128, 128, K/128, 128] tiles: [m, p, c, j] = wT[128c+p, 128m+j]
    K, M = wT.shape
    a = wT.reshape(K // 128, 128, M // 128, 128)
    return np.ascontiguousarray(a.transpose(2, 1, 0, 3))


def _pmajor(x):
    # [DIM, S] -> [128, 8, S]: (c p) s -> p c s
    S = x.shape[1]
    return np.ascontiguousarray(x.reshape(8, 128, S).transpose(1, 0, 2))


def _prep(inputs):
    """Returns (key, in_maps) for the 8 cores."""
    np32 = lambda x: np.asarray(x, dtype=np.float32)
    Q = np32(inputs["Q"]); K = np32(inputs["K"]); mask = np32(inputs["mask"])
    wq = np32(inputs["wq"]); bq = np32(inputs["bq"])
    wk = np32(inputs["wk"]); bk = np32(inputs["bk"])
    wv = np32(inputs["wv"]); bv = np32(inputs["bv"])
    wp = np32(inputs["wp"]); bp = np32(inputs["bp"])
    w1 = np32(inputs["w1"]); b1 = np32(inputs["b1"])
    w2 = np32(inputs["w2"]); b2 = np32(inputs["b2"])

    bp_eff = bp + wp @ bv          # fold v bias through the projection

    mask_ones = bool(np.all(mask == 1.0))
    bq_nz = bool(np.any(bq)); bk_nz = bool(np.any(bk))
    bp_nz = bool(np.any(bp_eff)); b2_nz = bool(np.any(b2))
    key = (mask_ones, bq_nz, bk_nz, bp_nz, b2_nz)

    wkey = tuple(
        (a.__array_interface__["data"][0], a.shape)
        for a in (wq, wk, wv, wp, w1, w2, b1)
    )
    cached = _host_cache.get("w")
    if cached is not None and cached[0] == wkey:
        wq_t, wk_t, wv_t, wp_t, w1_t, w2_t, b1_t = cached[1]
    else:
        wq_t = _tile_lhsT((wq * WS).T, 8, 8).astype(F8)
        wk_t = _tile_lhsT((wk * WS).T, 8, 8).astype(F8)
        wv_t = np.ascontiguousarray((wv * WS).T.reshape(8, 128, D)).astype(F8)
        wp_t = _tile_lhsT((wp * WS).T, 8, 8).astype(F8)
        w1_t = _tile_lhsT(w1.T, 16, 8).astype(BF)
        w2_t = _tile_lhsT(w2.T, 8, 16).astype(BF)
        b1_t = np.ascontiguousarray(b1.reshape(16, 128).T)
        _host_cache["w"] = (wkey, (wq_t, wk_t, wv_t, wp_t, w1_t, w2_t, b1_t))

    Kb_f8 = [_pmajor(K[b]).astype(F8) for b in range(B)]
    madd_t = [
        np.ascontiguousarray((-(1.0 - mask[b, 0]) * 10000.0).reshape(16, 128).T)
        for b in range(B)
    ]

    in_maps = []
    for c in range(NCORES):
        b, s = c // 4, c % 4
        sl = slice(QSL * s, QSL * s + QSL)
        Qb_pm = _pmajor(Q[b][:, sl])
        m = {
            "Kb": Kb_f8[b],
            "Qb": Qb_pm.astype(F8),
            "Qres": Qb_pm,
            "wq": wq_t, "wk": wk_t, "wv": wv_t, "wp": wp_t,
            "w1": w1_t, "w2": w2_t, "b1t": b1_t,
        }
        if bq_nz:
            m["bqt"] = np.ascontiguousarray((bq * WS).reshape(8, 128).T)
        if bk_nz:
            m["bkt"] = np.ascontiguousarray((bk * WS).reshape(8, 128).T)
        if bp_nz:
            m["bpt"] = np.ascontiguousarray(bp_eff.reshape(8, 128).T)
        if b2_nz:
            m["b2t"] = np.ascontiguousarray(b2.reshape(8, 128).T)
        if not mask_ones:
            m["maddt"] = madd_t[b]
        in_maps.append(m)
    return key, in_maps


def kernel(**inputs):
    key, in_maps = _prep(inputs)
    if key not in _nc_cache:
        _nc_cache[key] = _build_nc(*key)
    nc = _nc_cache[key]

    from concourse.bass_utils import run_bass_kernel_spmd

    res = run_bass_kernel_spmd(nc, in_maps, list(range(NCORES)))

    out = np.empty((B, DIM, SQ), np.float32)
    for c in range(NCORES):
        b, s = c // 4, c % 4
        out[b][:, QSL * s : QSL * s + QSL] = res.results[c]["out"]
    return out
